# revision 31
# baseline (speedup 1.0000x reference)
"""GQA attention with RoPE on 8 TRN2 NeuronCores (Bass/Tile, bf16 compute).

Sharding:
  - Projections are token-sharded: core c owns 512 tokens (batch0 chunk c,
    batch1 chunk 7-c), computes Q.T/K.T/V.T (feature-major) for them.
  - AllToAlls flip to head-sharded attention: core c receives its 4 query
    heads (4c..4c+3) + kv head c for ALL tokens. Causal attention work is
    then identical on every core (SPMD-uniform). The Q exchange is split by
    head pair so attention on pair 0 overlaps the rest of the Q projection.
  - Attention: scores computed transposed (S.T = K @ Q.T) so softmax'd probs
    feed the AV matmul without transposes; exp without max subtraction
    (scores are bounded ~|9| here); softmax denominators come free from a
    ones-column appended to V.
  - Per-batch AllToAlls return attention outputs to token-sharding; the
    output projection accumulates per-batch so batch-0 matmuls overlap the
    tail of attention. Each core emits y.T for its 512 tokens.
"""
import os
import numpy as np
import ml_dtypes

N_CORES = 8
B, L, D = 2, 2048, 2048
N_HEADS, KV_HEADS, HEAD_DIM = 32, 8, 64
THETA = 10000.0
KV_DIM = KV_HEADS * HEAD_DIM          # 512
TPC = B * L // N_CORES                # 512 tokens per core
CHUNK = L // N_CORES                  # 256 tokens per (batch, core)
DCH = D // 128                        # 16 contraction chunks
NKB = L // 128                        # 16 key blocks per batch
NEG = -1.0e9

_BUILT = {}


def _build_nc():
    import concourse.bacc as bacc
    import concourse.tile as tile
    from concourse import mybir
    from concourse.masks import make_identity

    f32 = mybir.dt.float32
    bf16 = mybir.dt.bfloat16

    nc = bacc.Bacc("TRN2", target_bir_lowering=False, debug=False,
                   num_devices=N_CORES)

    xT_ext = nc.dram_tensor("xT", [D, TPC], bf16, kind="ExternalInput")
    wqT_ext = nc.dram_tensor("wqT", [D, 2 * 128], bf16, kind="ExternalInput")
    wkvT_ext = nc.dram_tensor("wkvT", [D, 128], bf16, kind="ExternalInput")
    woT_ext = nc.dram_tensor("woT", [D, D], bf16, kind="ExternalInput")
    cos_ext = nc.dram_tensor("cosT", [128, B * L], bf16, kind="ExternalInput")
    sinN_ext = nc.dram_tensor("sinT", [128, B * L], bf16, kind="ExternalInput")
    mask_ext = nc.dram_tensor("maskT", [128, 2 * CHUNK], f32, kind="ExternalInput")
    yT_ext = nc.dram_tensor("yT", [D, TPC], f32, kind="ExternalOutput")

    rg = [list(range(N_CORES))]

    with tile.TileContext(nc) as tc:
        with tc.tile_pool(name="dram", bufs=1, space="DRAM") as dram, \
             tc.tile_pool(name="const", bufs=1) as const, \
             tc.tile_pool(name="big", bufs=1) as big:
            x_loc = dram.tile([D, TPC], bf16)
            x_all = dram.tile([8 * D, TPC], bf16, addr_space="Shared")
            ain = [[dram.tile([D // 2, CHUNK], bf16, name=f"ain{b}{p}",
                              tag=f"ain{b}{p}") for p in range(2)]
                   for b in range(2)]
            aout = [[dram.tile([D // 2, CHUNK], bf16, name=f"aout{b}{p}",
                               tag=f"aout{b}{p}") for p in range(2)]
                    for b in range(2)]

            cos_sb = const.tile([128, B * L], bf16, tag="cos")
            sin_sb = const.tile([128, B * L], bf16, tag="sin")
            mask_sb = const.tile([128, 2 * CHUNK], f32, tag="mask")
            ones_sb = const.tile([128, 128], f32, tag="ones")
            ident = const.tile([64, 64], bf16, tag="ident")
            nc.gpsimd.dma_start(out=cos_sb[:], in_=cos_ext[:])
            nc.gpsimd.dma_start(out=sin_sb[:], in_=sinN_ext[:])
            nc.gpsimd.dma_start(out=mask_sb[:], in_=mask_ext[:])
            nc.any.memset(ones_sb[:], 1.0)
            make_identity(nc, ident[:])


            # x AllGather fires immediately; projections are head-sharded
            # (each core holds only its own weight slices, sliced on host)
            nc.sync.dma_start(out=x_loc[:], in_=xT_ext[:])
            nc.gpsimd.collective_compute(
                "AllGather", mybir.AluOpType.bypass, replica_groups=rg,
                ins=[x_loc[:].opt()], outs=[x_all[:].opt()])
            xg = x_all[:].rearrange("(i d) t -> i d t", d=D)   # [8, 2048, 512]

            # ---------------- fused projections + attention prep ----------------
            with tc.tile_pool(name="att_sb", bufs=1) as att_sb, \
                 tc.tile_pool(name="vsb", bufs=1) as vsb, \
                 tc.tile_pool(name="qsb", bufs=1) as qsb:
                # persistent attention operand tiles
                kdup = [att_sb.tile([128, L], bf16, name=f"kdup{b}", tag=f"kdup{b}")
                        for b in range(2)]
                va = [att_sb.tile([64, L], bf16, name=f"va{b}", tag=f"va{b}")
                      for b in range(2)]
                vones = [vsb.tile([128, NKB * 65], bf16, name=f"vones{b}",
                                  tag=f"vones{b}") for b in range(2)]
                qp = [[qsb.tile([128, L], bf16, name=f"qp{p}{b}", tag=f"qp{p}{b}")
                       for b in range(2)] for p in range(2)]
                for b in range(2):
                    nc.any.memset(vones[b][:], 1.0)

                with tc.tile_pool(name="pp", bufs=4, space="PSUM") as pp, \
                     tc.tile_pool(name="vt", bufs=2, space="PSUM") as vt, \
                     tc.tile_pool(name="rp", bufs=4) as rp, \
                     tc.tile_pool(name="xp", bufs=2) as xp, \
                     tc.tile_pool(name="wsb", bufs=1) as wsb:
                    wkv_sb = wsb.tile([128, DCH * 128], bf16, tag="wkv")
                    nc.sync.dma_start(
                        out=wkv_sb[:].rearrange("p (i e) -> p i e", i=DCH),
                        in_=wkvT_ext.rearrange("(i p) e -> p i e", p=128))
                    wq_sb = wsb.tile([128, DCH * 256], bf16, tag="wqs")
                    nc.sync.dma_start(
                        out=wq_sb[:].rearrange("p (i e) -> p i e", i=DCH),
                        in_=wqT_ext.rearrange("(i p) e -> p i e", p=128))

                    def rope_cols(ps, prow, c_out_list):
                        """RoPE rows [prow, prow+64) of psum tile for global
                        token block i; write bf16 results to c_out_list =
                        [(dst_ap_b0, dst_ap_b1)] entries (cols 0:256 -> b0,
                        256:512 -> b1)."""
                        pass

                    for i in range(8):
                        xblk = xp.tile([128, DCH * TPC], bf16, tag="xblk")
                        for dc4 in range(4):
                            nc.sync.dma_start(
                                out=xblk[:, TPC * 4 * dc4:TPC * 4 * (dc4 + 1)]
                                .rearrange("p (c t) -> p c t", c=4),
                                in_=xg[i].rearrange("(c p) t -> p c t", p=128)[:, 4 * dc4:4 * (dc4 + 1), :])
                        gcol = [slice(CHUNK * i, CHUNK * (i + 1)),
                                slice(CHUNK * (7 - i), CHUNK * (8 - i))]
                        tcos = cos_sb[:, 512 * i:512 * (i + 1)]
                        tsin = sin_sb[:, 512 * i:512 * (i + 1)]

                        # K (rows 0:64) + V (rows 64:128) projection
                        ps = pp.tile([128, TPC], f32, tag="proj")
                        for dc in range(DCH):
                            nc.tensor.matmul(
                                ps[:], wkv_sb[:, 128 * dc:128 * (dc + 1)],
                                xblk[:, TPC * dc:TPC * (dc + 1)],
                                start=(dc == 0), stop=(dc == DCH - 1))
                        # V.T -> va (ACT partition shift 64->0)
                        for b in range(2):
                            nc.scalar.copy(va[b][:, gcol[b]],
                                           ps[64:128, CHUNK * b:CHUNK * (b + 1)])
                        # K rope: bf16 evict + ACT swap copies, bf16 DVE math
                        kb_t = rp.tile([64, TPC], bf16, tag="kb")
                        nc.scalar.copy(kb_t[:], ps[0:64, :])
                        ksw = rp.tile([64, TPC], bf16, tag="ksw")
                        nc.scalar.copy(ksw[0:32, :], ps[32:64, :])
                        nc.scalar.copy(ksw[32:64, :], ps[0:32, :])
                        t1 = rp.tile([64, TPC], bf16, tag="t1")
                        nc.vector.tensor_mul(t1[:], kb_t[:], tcos[0:64, :])
                        t2 = rp.tile([64, TPC], bf16, tag="t2")
                        nc.vector.tensor_mul(t2[:], ksw[:], tsin[0:64, :])
                        for b in range(2):
                            cs = slice(CHUNK * b, CHUNK * (b + 1))
                            nc.vector.tensor_add(kdup[b][0:64, gcol[b]],
                                                 t1[:, cs], t2[:, cs])
                            nc.vector.tensor_add(kdup[b][64:128, gcol[b]],
                                                 t1[:, cs], t2[:, cs])

                        # Q pairs
                        for p01 in range(2):
                            psq = pp.tile([128, TPC], f32, tag="proj")
                            for dc in range(DCH):
                                nc.tensor.matmul(
                                    psq[:],
                                    wq_sb[:, 256 * dc + 128 * p01:
                                          256 * dc + 128 * p01 + 128],
                                    xblk[:, TPC * dc:TPC * (dc + 1)],
                                    start=(dc == 0), stop=(dc == DCH - 1))
                            qb_t = rp.tile([128, TPC], bf16, tag="qb")
                            nc.scalar.copy(qb_t[:], psq[:])
                            qsw = rp.tile([128, TPC], bf16, tag="qsw")
                            for h in range(2):
                                b0r = 64 * h
                                nc.scalar.copy(qsw[b0r:b0r + 32, :],
                                               psq[b0r + 32:b0r + 64, :])
                                nc.scalar.copy(qsw[b0r + 32:b0r + 64, :],
                                               psq[b0r:b0r + 32, :])
                            q1 = rp.tile([128, TPC], bf16, tag="q1")
                            nc.vector.tensor_mul(q1[:], qb_t[:], tcos[:])
                            q2 = rp.tile([128, TPC], bf16, tag="q2")
                            nc.vector.tensor_mul(q2[:], qsw[:], tsin[:])
                            for b in range(2):
                                cs = slice(CHUNK * b, CHUNK * (b + 1))
                                nc.vector.tensor_add(qp[p01][b][:, gcol[b]],
                                                     q1[:, cs], q2[:, cs])

                    # V transposes into vones (both batches)
                    for b in range(2):
                        for k in range(NKB):
                            tp = vt.tile([128, 64], bf16, tag="vt")
                            nc.tensor.transpose(
                                tp[:], va[b][:, 128 * k:128 * (k + 1)], ident[:])
                            nc.scalar.copy(vones[b][:, 65 * k:65 * k + 64], tp[:])

                with tc.tile_pool(name="ps_s", bufs=2, space="PSUM") as ps_s, \
                     tc.tile_pool(name="ps_av", bufs=4, space="PSUM") as ps_av, \
                     tc.tile_pool(name="pt_sb", bufs=6) as pt_sb, \
                     tc.tile_pool(name="nrm", bufs=4) as nrm, \
                     tc.tile_pool(name="wo_pool", bufs=16) as wo_pool, \
                     tc.tile_pool(name="yo", bufs=4) as yo, \
                     tc.tile_pool(name="ao", bufs=1) as ao:
                    # output-projection resources; weight prefetch streams in
                    # during attention on the gpsimd DMA queue
                    wo_ts = []
                    for dt in range(DCH):
                        wo_t = wo_pool.tile([128, DCH * 128], bf16, tag="wo",
                                            name=f"wo{dt}")
                        nc.sync.dma_start(
                            out=wo_t[:].rearrange("p (i e) -> p i e", i=DCH),
                            in_=woT_ext.rearrange("(i p) e -> p i e", p=128)[:, :, 128 * dt:128 * dt + 128])
                        wo_ts.append(wo_t)
                    aout_sb = ao.tile([128, DCH * TPC], bf16, tag="aout")

                    def emit_oproj(bb):
                        for j in range(DCH):
                            nc.sync.dma_start(
                                out=aout_sb[:, TPC * j + CHUNK * bb:
                                            TPC * j + CHUNK * (bb + 1)],
                                in_=aout[bb][j % 2][128 * (j // 2):128 * (j // 2 + 1), :])
                        for dt in range(DCH):
                            ps = ps_av.tile([128, CHUNK], f32, tag="av",
                                            name=f"po{bb}_{dt}")
                            for jj in range(DCH):
                                nc.tensor.matmul(
                                    ps[:],
                                    wo_ts[dt][:, 128 * jj:128 * (jj + 1)],
                                    aout_sb[:, TPC * jj + CHUNK * bb:
                                            TPC * jj + CHUNK * (bb + 1)],
                                    start=(jj == 0), stop=(jj == DCH - 1))
                            yv = yo.tile([128, CHUNK], f32, tag="y")
                            nc.scalar.copy(yv[:], ps[:])
                            nc.sync.dma_start(
                                out=yT_ext[128 * dt:128 * (dt + 1),
                                           CHUNK * bb:CHUNK * (bb + 1)],
                                in_=yv[:])
                    for b in range(2):
                        for p01 in range(2):
                            if b == 1 and p01 == 1:
                                emit_oproj(0)
                            qt = qp[p01][b]
                            for j in range(8):
                                qc = slice(CHUNK * j, CHUNK * (j + 1))
                                avt = [ps_av.tile([65, CHUNK], f32,
                                                   name=f"av{_h}", tag="av")
                                       for _h in range(2)]
                                avs = [avt[0][:], avt[1][:]]
                                nkb2 = 2 * (j + 1)          # 128-key blocks
                                ntile = (nkb2 + 3) // 4
                                pts = {}

                                def emit_av(k4i):
                                    nsl_i = min(4, nkb2 - 4 * k4i)
                                    for half in range(2):
                                        for s in range(nsl_i):
                                            kb = 4 * k4i + s
                                            nc.tensor.matmul(
                                                avs[half],
                                                vones[b][:, 65 * kb:65 * kb + 65],
                                                pts[k4i][half][:, 256 * s:256 * (s + 1)],
                                                start=(kb == 0),
                                                stop=(kb == nkb2 - 1))

                                for k4 in range(ntile):
                                    nsl = min(4, nkb2 - 4 * k4)   # 2 or 4 slots
                                    pts[k4] = []
                                    for half in range(2):
                                        h0 = 64 * half
                                        ssp = ps_s.tile([128, 1024], f32, tag="s")
                                        for s in range(nsl):
                                            kb = 4 * k4 + s
                                            nc.tensor.matmul(
                                                ssp[:, 256 * s:256 * (s + 1)],
                                                kdup[b][h0:h0 + 64,
                                                        128 * kb:128 * (kb + 1)],
                                                qt[h0:h0 + 64, qc],
                                                start=True, stop=True)
                                        pt = pt_sb.tile([128, 1024], bf16, tag="pt")
                                        ndiag = 2 if 4 * k4 + nsl == nkb2 else 0
                                        nnd = nsl - ndiag
                                        if nnd:
                                            nc.scalar.activation(
                                                pt[:, 0:256 * nnd],
                                                ssp[:, 0:256 * nnd],
                                                mybir.ActivationFunctionType.Exp,
                                                scale=0.125)
                                        if ndiag:
                                            c0 = 256 * nnd
                                            sm = nrm.tile([128, 512], f32, tag="sm")
                                            nc.vector.scalar_tensor_tensor(
                                                sm[:], ssp[:, c0:c0 + 512], 0.125,
                                                mask_sb[:],
                                                op0=mybir.AluOpType.mult,
                                                op1=mybir.AluOpType.add)
                                            nc.scalar.activation(
                                                pt[:, c0:c0 + 512], sm[:],
                                                mybir.ActivationFunctionType.Exp)
                                        pts[k4].append(pt)
                                    if k4 >= 1:
                                        emit_av(k4 - 1)
                                emit_av(ntile - 1)
                                blk = j if b == 0 else 7 - j
                                an = nrm.tile([128, CHUNK], bf16, tag="an")
                                for half in range(2):
                                    lrow = nrm.tile([1, CHUNK], f32, tag="lrow")
                                    nc.vector.tensor_copy(lrow[:], avs[half][64:65, :])
                                    linv = nrm.tile([1, CHUNK], f32, tag="linv")
                                    nc.vector.reciprocal_approx_fast(
                                        out=linv[:], in_=lrow[:])
                                    bcs = nrm.tile([64, CHUNK], f32, tag="bcs")
                                    nc.gpsimd.partition_broadcast(
                                        bcs[:], linv[0:1, :], channels=64)
                                    nc.vector.tensor_mul(
                                        an[64 * half:64 * half + 64, :],
                                        avs[half][0:64, :], bcs[:])
                                nc.sync.dma_start(
                                    out=ain[b][p01][128 * blk:128 * blk + 128, :],
                                    in_=an[:])
                            nc.gpsimd.collective_compute(
                                "AllToAll", mybir.AluOpType.bypass,
                                replica_groups=rg,
                                ins=[ain[b][p01][:].opt()],
                                outs=[aout[b][p01][:].opt()])
                    emit_oproj(1)

    nc.compile()
    return nc


def _host_inputs(x, wq, wk, wv, wo):
    bf = ml_dtypes.bfloat16
    woT = np.ascontiguousarray(wo.T).astype(bf)
    wqT_f = np.ascontiguousarray(wq.T).astype(bf)   # [D, 2048]
    wkT_f = np.ascontiguousarray(wk.T).astype(bf)   # [D, 512]
    wvT_f = np.ascontiguousarray(wv.T).astype(bf)

    q_idx = np.arange(CHUNK)
    k_idx = np.arange(128)
    m0 = np.where(k_idx[:, None] <= q_idx[None, :], 0.0, NEG).astype(np.float32)
    m1 = np.where(k_idx[:, None] + 128 <= q_idx[None, :], 0.0, NEG).astype(np.float32)
    mask = np.concatenate([m0, m1], axis=1)

    freqs = 1.0 / (THETA ** (np.arange(0, HEAD_DIM, 2, dtype=np.float32) / HEAD_DIM))
    # global rope tables in gathered-block order: block i = [b0 chunk i; b1 chunk 7-i]
    pos = np.concatenate([
        np.concatenate([np.arange(CHUNK * i, CHUNK * (i + 1)),
                        np.arange(CHUNK * (7 - i), CHUNK * (8 - i))])
        for i in range(8)]).astype(np.float32)              # [4096]
    ph = np.outer(freqs, pos)                               # [32, 4096]
    cos64 = np.cos(np.concatenate([ph, ph], axis=0))        # [64, 4096]
    sin64 = np.sin(np.concatenate([ph, ph], axis=0))
    sgn = np.where(np.arange(64) < 32, -1.0, 1.0)[:, None].astype(np.float32)
    cosT = np.concatenate([cos64, cos64], axis=0).astype(bf)     # [128, 4096]
    sinT = np.concatenate([sin64 * sgn, sin64 * sgn], axis=0).astype(bf)

    in_maps = []
    for c in range(N_CORES):
        a_sl = slice(CHUNK * c, CHUNK * (c + 1))
        b_sl = slice(CHUNK * (7 - c), CHUNK * (8 - c))
        xT = np.concatenate([x[0, a_sl, :].T, x[1, b_sl, :].T], axis=1)
        wqT_my = np.ascontiguousarray(wqT_f[:, 256 * c:256 * (c + 1)])
        wkvT_my = np.ascontiguousarray(np.concatenate(
            [wkT_f[:, 64 * c:64 * (c + 1)], wvT_f[:, 64 * c:64 * (c + 1)]], axis=1))
        in_maps.append({
            "xT": np.ascontiguousarray(xT).astype(bf),
            "wqT": wqT_my, "wkvT": wkvT_my, "woT": woT,
            "cosT": cosT, "sinT": sinT, "maskT": mask,
        })
    return in_maps


def kernel(x, wq, wk, wv, wo):
    from concourse.bass_utils import run_bass_kernel_spmd

    if "nc" not in _BUILT:
        _BUILT["nc"] = _build_nc()
    nc = _BUILT["nc"]

    in_maps = _host_inputs(np.asarray(x), np.asarray(wq), np.asarray(wk),
                           np.asarray(wv), np.asarray(wo))
    trace = bool(os.environ.get("BASS_KERNEL_TRACE"))
    res = run_bass_kernel_spmd(nc, in_maps, core_ids=list(range(N_CORES)),
                               trace=trace)
    kernel.last_exec_time_ns = res.exec_time_ns
    kernel.last_results = res

    y = np.empty((B, L, D), dtype=np.float32)
    for c in range(N_CORES):
        yT = res.results[c]["yT"]
        y[0, CHUNK * c: CHUNK * (c + 1), :] = yT[:, 0:CHUNK].T
        y[1, CHUNK * (7 - c): CHUNK * (8 - c), :] = yT[:, CHUNK:].T
    return y


# revision 33
# speedup vs baseline: 1.1145x; 1.1145x over previous
"""GQA attention with RoPE on 8 TRN2 NeuronCores (Bass/Tile, bf16 compute).

Sharding:
  - Projections are token-sharded: core c owns 512 tokens (batch0 chunk c,
    batch1 chunk 7-c), computes Q.T/K.T/V.T (feature-major) for them.
  - AllToAlls flip to head-sharded attention: core c receives its 4 query
    heads (4c..4c+3) + kv head c for ALL tokens. Causal attention work is
    then identical on every core (SPMD-uniform). The Q exchange is split by
    head pair so attention on pair 0 overlaps the rest of the Q projection.
  - Attention: scores computed transposed (S.T = K @ Q.T) so softmax'd probs
    feed the AV matmul without transposes; exp without max subtraction
    (scores are bounded ~|9| here); softmax denominators come free from a
    ones-column appended to V.
  - Per-batch AllToAlls return attention outputs to token-sharding; the
    output projection accumulates per-batch so batch-0 matmuls overlap the
    tail of attention. Each core emits y.T for its 512 tokens.
"""
import os
import numpy as np
import ml_dtypes

N_CORES = 8
B, L, D = 2, 2048, 2048
N_HEADS, KV_HEADS, HEAD_DIM = 32, 8, 64
THETA = 10000.0
KV_DIM = KV_HEADS * HEAD_DIM          # 512
TPC = B * L // N_CORES                # 512 tokens per core
CHUNK = L // N_CORES                  # 256 tokens per (batch, core)
DCH = D // 128                        # 16 contraction chunks
NKB = L // 128                        # 16 key blocks per batch
NEG = -1.0e9

_BUILT = {}


def _build_nc():
    import concourse.bacc as bacc
    import concourse.tile as tile
    from concourse import mybir
    from concourse.masks import make_identity

    f32 = mybir.dt.float32
    bf16 = mybir.dt.bfloat16

    nc = bacc.Bacc("TRN2", target_bir_lowering=False, debug=False,
                   num_devices=N_CORES)

    xT_ext = nc.dram_tensor("xT", [D, TPC], bf16, kind="ExternalInput")
    wqT_ext = nc.dram_tensor("wqT", [D, D], bf16, kind="ExternalInput")
    wkT_ext = nc.dram_tensor("wkT", [D, KV_DIM], bf16, kind="ExternalInput")
    wvT_ext = nc.dram_tensor("wvT", [D, KV_DIM], bf16, kind="ExternalInput")
    woT_ext = nc.dram_tensor("woT", [D, D], bf16, kind="ExternalInput")
    cos_ext = nc.dram_tensor("cosT", [128, TPC], bf16, kind="ExternalInput")
    sinN_ext = nc.dram_tensor("sinT", [128, TPC], bf16, kind="ExternalInput")
    mask_ext = nc.dram_tensor("maskT", [128, 2 * CHUNK], f32, kind="ExternalInput")
    yT_ext = nc.dram_tensor("yT", [D, TPC], f32, kind="ExternalOutput")

    rg = [list(range(N_CORES))]

    with tile.TileContext(nc) as tc:
        with tc.tile_pool(name="dram", bufs=1, space="DRAM") as dram, \
             tc.tile_pool(name="const", bufs=1) as const, \
             tc.tile_pool(name="big", bufs=1) as big:
            kv_local = dram.tile([8 * 128, TPC], bf16)
            kv_out = dram.tile([8 * 128, TPC], bf16)
            q_local = [dram.tile([8 * 128, TPC], bf16, name=f"ql{p}", tag=f"ql{p}")
                       for p in range(2)]
            q_out = [dram.tile([8 * 128, TPC], bf16, name=f"qo{p}", tag=f"qo{p}")
                     for p in range(2)]
            ain = [[dram.tile([D // 2, CHUNK], bf16, name=f"ain{b}{p}",
                              tag=f"ain{b}{p}") for p in range(2)]
                   for b in range(2)]
            aout = [[dram.tile([D // 2, CHUNK], bf16, name=f"aout{b}{p}",
                               tag=f"aout{b}{p}") for p in range(2)]
                    for b in range(2)]

            cos_sb = const.tile([128, TPC], bf16, tag="cos")
            sin_sb = const.tile([128, TPC], bf16, tag="sin")
            mask_sb = const.tile([128, 2 * CHUNK], f32, tag="mask")
            ones_sb = const.tile([128, 128], f32, tag="ones")
            ident = const.tile([64, 64], bf16, tag="ident")
            nc.gpsimd.dma_start(out=cos_sb[:], in_=cos_ext[:])
            nc.gpsimd.dma_start(out=sin_sb[:], in_=sinN_ext[:])
            nc.gpsimd.dma_start(out=mask_sb[:], in_=mask_ext[:])
            nc.any.memset(ones_sb[:], 1.0)
            make_identity(nc, ident[:])
            warm_in = dram.tile([8, 16], f32, tag="warm_in")
            warm_out = dram.tile([8, 16], f32, tag="warm_out")
            nc.sync.dma_start(out=warm_in[:], in_=mask_ext[0:8, 0:16])
            nc.gpsimd.collective_compute(
                "AllToAll", mybir.AluOpType.bypass, replica_groups=rg,
                ins=[warm_in[:].opt()], outs=[warm_out[:].opt()])


            # ---------------- projections (token-sharded) ----------------
            def rope_evict(ps, rp, out_bf):
                """RoPE a [128, TPC] feature-major psum tile -> bf16 sbuf.
                Evict to bf16 first (ACT), then bf16 DVE ops (fast mode);
                rotate-half via partition-shifted SBUF reads."""
                qb = rp.tile([128, TPC], bf16, tag="qb")
                nc.scalar.copy(qb[:], ps[:])
                t1 = rp.tile([128, TPC], bf16, tag="t1")
                nc.vector.tensor_mul(t1[:], qb[:], cos_sb[:])
                t2 = rp.tile([128, TPC], bf16, tag="t2")
                for h in range(2):
                    b0 = 64 * h
                    nc.vector.tensor_mul(t2[b0:b0 + 32, :],
                                         ps[b0 + 32:b0 + 64, :],
                                         sin_sb[b0:b0 + 32, :])
                    nc.vector.tensor_mul(t2[b0 + 32:b0 + 64, :],
                                         ps[b0:b0 + 32, :],
                                         sin_sb[b0 + 32:b0 + 64, :])
                nc.vector.tensor_add(out_bf[:], t1[:], t2[:])

            with tc.tile_pool(name="pp", bufs=4, space="PSUM") as pp, \
                 tc.tile_pool(name="rp", bufs=4) as rp, \
                 tc.tile_pool(name="acts", bufs=1) as acts:
                # chunked loads so first matmuls start early; wq preloaded in
                # full so HBM is quiet when the AllToAlls run
                xT_sb = acts.tile([128, DCH * TPC], bf16, tag="xT")
                wk_sb = acts.tile([128, DCH * KV_DIM], bf16, tag="wk")
                wv_sb = acts.tile([128, DCH * KV_DIM], bf16, tag="wv")
                wq_sb = acts.tile([128, DCH * 128 * DCH], bf16, tag="wqall")
                for i in range(DCH):
                    nc.sync.dma_start(out=wk_sb[:, KV_DIM * i:KV_DIM * (i + 1)],
                                      in_=wkT_ext[128 * i:128 * (i + 1), :])
                    nc.sync.dma_start(out=wv_sb[:, KV_DIM * i:KV_DIM * (i + 1)],
                                      in_=wvT_ext[128 * i:128 * (i + 1), :])
                    nc.gpsimd.dma_start(out=xT_sb[:, TPC * i:TPC * (i + 1)],
                                        in_=xT_ext[128 * i:128 * (i + 1), :])
                for p01 in range(2):
                    for j in range(8):
                        t = 2 * j + p01
                        nc.gpsimd.dma_start(
                            out=wq_sb[:, 2048 * t:2048 * (t + 1)]
                            .rearrange("p (i e) -> p i e", i=DCH),
                            in_=wqT_ext.rearrange("(i p) e -> p i e", p=128)[:, :, 128 * t:128 * t + 128])
                # K/V projections first -> A2A_kv is in flight during Q proj
                for u in range(KV_DIM // 128):
                    for which, wsb in (("k", wk_sb), ("v", wv_sb)):
                        ps = pp.tile([128, TPC], f32, tag="proj")
                        for i in range(DCH):
                            nc.tensor.matmul(
                                ps[:],
                                wsb[:, KV_DIM * i + 128 * u: KV_DIM * i + 128 * u + 128],
                                xT_sb[:, TPC * i: TPC * (i + 1)],
                                start=(i == 0), stop=(i == DCH - 1))
                        ob = rp.tile([128, TPC], bf16, tag="projb")
                        if which == "k":
                            rope_evict(ps[:], rp, ob)
                        else:
                            nc.scalar.copy(ob[:], ps[:])
                        off = 64 if which == "v" else 0
                        nc.sync.dma_start(
                            out=kv_local[128 * (2 * u) + off: 128 * (2 * u) + off + 64, :],
                            in_=ob[0:64, :])
                        nc.sync.dma_start(
                            out=kv_local[128 * (2 * u + 1) + off: 128 * (2 * u + 1) + off + 64, :],
                            in_=ob[64:128, :])
                nc.gpsimd.collective_compute(
                    "AllToAll", mybir.AluOpType.bypass, replica_groups=rg,
                    ins=[kv_local[:].opt()], outs=[kv_out[:].opt()])

                # Q projection: even e-tiles (head pair 0 of each group) first
                for p01 in range(2):
                    for j in range(8):
                        t = 2 * j + p01
                        ps = pp.tile([128, TPC], f32, tag="proj")
                        for i in range(DCH):
                            nc.tensor.matmul(
                                ps[:], wq_sb[:, 2048 * t + 128 * i:2048 * t + 128 * (i + 1)],
                                xT_sb[:, TPC * i: TPC * (i + 1)],
                                start=(i == 0), stop=(i == DCH - 1))
                        ob = rp.tile([128, TPC], bf16, tag="projb")
                        rope_evict(ps[:], rp, ob)
                        nc.sync.dma_start(out=q_local[p01][128 * j:128 * (j + 1), :],
                                          in_=ob[:])
                    nc.gpsimd.collective_compute(
                        "AllToAll", mybir.AluOpType.bypass, replica_groups=rg,
                        ins=[q_local[p01][:].opt()], outs=[q_out[p01][:].opt()])

            # ---------------- attention (head-sharded) ----------------
            kv_blocks = kv_out[:].rearrange("(i r) t -> i r t", r=128)

            with tc.tile_pool(name="att_sb", bufs=2) as att_sb, \
                 tc.tile_pool(name="vsb", bufs=1) as vsb:
                with tc.tile_pool(name="vt", bufs=2, space="PSUM") as vt:
                    vones, kdup = [], []
                    for b in range(2):
                        va = att_sb.tile([64, L], bf16, tag="va")
                        blk_src = kv_blocks[:, 64:128, CHUNK * b: CHUNK * (b + 1)] \
                            .rearrange("i r t -> r i t")
                        if b == 1:
                            blk_src = blk_src[:, ::-1, :]
                        nc.sync.dma_start(
                            out=va[:].rearrange("r (i t) -> r i t", i=8),
                            in_=blk_src)
                        vo = vsb.tile([128, NKB * 65], bf16, name=f"vones{b}",
                                      tag=f"vones{b}")
                        nc.any.memset(vo[:], 1.0)
                        for k in range(NKB):
                            tp = vt.tile([128, 64], bf16, tag="vt")
                            nc.tensor.transpose(
                                tp[:], va[:, 128 * k: 128 * (k + 1)], ident[:])
                            nc.scalar.copy(vo[:, 65 * k: 65 * k + 64], tp[:])
                        vones.append(vo)
                        kd = att_sb.tile([128, L], bf16, name=f"kdup{b}",
                                         tag=f"kdup{b}")
                        ksrc = kv_blocks[:, 0:64, CHUNK * b: CHUNK * (b + 1)] \
                            .rearrange("i r t -> r i t")
                        if b == 1:
                            ksrc = ksrc[:, ::-1, :]
                        for half in range(2):
                            nc.sync.dma_start(
                                out=kd[64 * half: 64 * half + 64, :]
                                .rearrange("r (i t) -> r i t", i=8),
                                in_=ksrc)
                        kdup.append(kd)

                with tc.tile_pool(name="ps_s", bufs=2, space="PSUM") as ps_s, \
                     tc.tile_pool(name="ps_av", bufs=4, space="PSUM") as ps_av, \
                     tc.tile_pool(name="pt_sb", bufs=6) as pt_sb, \
                     tc.tile_pool(name="nrm", bufs=4) as nrm, \
                     tc.tile_pool(name="wo_pool", bufs=16) as wo_pool, \
                     tc.tile_pool(name="yo", bufs=4) as yo, \
                     tc.tile_pool(name="ao", bufs=1) as ao, \
                     tc.tile_pool(name="qpool", bufs=2) as qpool:
                    # output-projection resources; weight prefetch streams in
                    # during attention on the gpsimd DMA queue
                    wo_ts = []
                    for dt in range(DCH):
                        wo_t = wo_pool.tile([128, DCH * 128], bf16, tag="wo",
                                            name=f"wo{dt}")
                        nc.sync.dma_start(
                            out=wo_t[:].rearrange("p (i e) -> p i e", i=DCH),
                            in_=woT_ext.rearrange("(i p) e -> p i e", p=128)[:, :, 128 * dt:128 * dt + 128])
                        wo_ts.append(wo_t)
                    aout_sb = ao.tile([128, DCH * TPC], bf16, tag="aout")

                    def emit_oproj(bb):
                        for j in range(DCH):
                            nc.sync.dma_start(
                                out=aout_sb[:, TPC * j + CHUNK * bb:
                                            TPC * j + CHUNK * (bb + 1)],
                                in_=aout[bb][j % 2][128 * (j // 2):128 * (j // 2 + 1), :])
                        for dt in range(DCH):
                            ps = ps_av.tile([128, CHUNK], f32, tag="av",
                                            name=f"po{bb}_{dt}")
                            for jj in range(DCH):
                                nc.tensor.matmul(
                                    ps[:],
                                    wo_ts[dt][:, 128 * jj:128 * (jj + 1)],
                                    aout_sb[:, TPC * jj + CHUNK * bb:
                                            TPC * jj + CHUNK * (bb + 1)],
                                    start=(jj == 0), stop=(jj == DCH - 1))
                            yv = yo.tile([128, CHUNK], f32, tag="y")
                            nc.scalar.copy(yv[:], ps[:])
                            nc.sync.dma_start(
                                out=yT_ext[128 * dt:128 * (dt + 1),
                                           CHUNK * bb:CHUNK * (bb + 1)],
                                in_=yv[:])
                    for b in range(2):
                        for p01 in range(2):
                            if b == 1 and p01 == 1:
                                emit_oproj(0)
                            qb = q_out[p01][:].rearrange("(i r) t -> i r t", r=128)
                            qt = qpool.tile([128, L], bf16, tag="qp")
                            qsrc = qb[:, :, CHUNK * b: CHUNK * (b + 1)] \
                                .rearrange("i r t -> r i t")
                            if b == 1:
                                qsrc = qsrc[:, ::-1, :]
                            nc.sync.dma_start(
                                out=qt[:].rearrange("r (i t) -> r i t", i=8),
                                in_=qsrc)
                            for j in range(8):
                                qc = slice(CHUNK * j, CHUNK * (j + 1))
                                avt = [ps_av.tile([65, CHUNK], f32,
                                                   name=f"av{_h}", tag="av")
                                       for _h in range(2)]
                                avs = [avt[0][:], avt[1][:]]
                                nkb2 = 2 * (j + 1)          # 128-key blocks
                                ntile = (nkb2 + 3) // 4
                                pts = {}

                                def emit_av(k4i):
                                    nsl_i = min(4, nkb2 - 4 * k4i)
                                    for half in range(2):
                                        for s in range(nsl_i):
                                            kb = 4 * k4i + s
                                            nc.tensor.matmul(
                                                avs[half],
                                                vones[b][:, 65 * kb:65 * kb + 65],
                                                pts[k4i][half][:, 256 * s:256 * (s + 1)],
                                                start=(kb == 0),
                                                stop=(kb == nkb2 - 1))

                                for k4 in range(ntile):
                                    nsl = min(4, nkb2 - 4 * k4)   # 2 or 4 slots
                                    pts[k4] = []
                                    for half in range(2):
                                        h0 = 64 * half
                                        ssp = ps_s.tile([128, 1024], f32, tag="s")
                                        for s in range(nsl):
                                            kb = 4 * k4 + s
                                            nc.tensor.matmul(
                                                ssp[:, 256 * s:256 * (s + 1)],
                                                kdup[b][h0:h0 + 64,
                                                        128 * kb:128 * (kb + 1)],
                                                qt[h0:h0 + 64, qc],
                                                start=True, stop=True)
                                        pt = pt_sb.tile([128, 1024], bf16, tag="pt")
                                        ndiag = 2 if 4 * k4 + nsl == nkb2 else 0
                                        nnd = nsl - ndiag
                                        if nnd:
                                            nc.scalar.activation(
                                                pt[:, 0:256 * nnd],
                                                ssp[:, 0:256 * nnd],
                                                mybir.ActivationFunctionType.Exp,
                                                scale=0.125)
                                        if ndiag:
                                            c0 = 256 * nnd
                                            sm = nrm.tile([128, 512], f32, tag="sm")
                                            nc.vector.scalar_tensor_tensor(
                                                sm[:], ssp[:, c0:c0 + 512], 0.125,
                                                mask_sb[:],
                                                op0=mybir.AluOpType.mult,
                                                op1=mybir.AluOpType.add)
                                            nc.scalar.activation(
                                                pt[:, c0:c0 + 512], sm[:],
                                                mybir.ActivationFunctionType.Exp)
                                        pts[k4].append(pt)
                                    if k4 >= 1:
                                        emit_av(k4 - 1)
                                emit_av(ntile - 1)
                                blk = j if b == 0 else 7 - j
                                an = nrm.tile([128, CHUNK], bf16, tag="an")
                                for half in range(2):
                                    lrow = nrm.tile([1, CHUNK], f32, tag="lrow")
                                    nc.vector.tensor_copy(lrow[:], avs[half][64:65, :])
                                    linv = nrm.tile([1, CHUNK], f32, tag="linv")
                                    nc.vector.reciprocal_approx_fast(
                                        out=linv[:], in_=lrow[:])
                                    bcs = nrm.tile([64, CHUNK], f32, tag="bcs")
                                    nc.gpsimd.partition_broadcast(
                                        bcs[:], linv[0:1, :], channels=64)
                                    nc.vector.tensor_mul(
                                        an[64 * half:64 * half + 64, :],
                                        avs[half][0:64, :], bcs[:])
                                nc.sync.dma_start(
                                    out=ain[b][p01][128 * blk:128 * blk + 128, :],
                                    in_=an[:])
                            nc.gpsimd.collective_compute(
                                "AllToAll", mybir.AluOpType.bypass,
                                replica_groups=rg,
                                ins=[ain[b][p01][:].opt()],
                                outs=[aout[b][p01][:].opt()])
                    emit_oproj(1)

    nc.compile()
    return nc


def _host_inputs(x, wq, wk, wv, wo):
    bf = ml_dtypes.bfloat16
    wqT = np.ascontiguousarray(wq.T).astype(bf)
    wkT = np.ascontiguousarray(wk.T).astype(bf)
    wvT = np.ascontiguousarray(wv.T).astype(bf)
    woT = np.ascontiguousarray(wo.T).astype(bf)

    q_idx = np.arange(CHUNK)
    k_idx = np.arange(128)
    m0 = np.where(k_idx[:, None] <= q_idx[None, :], 0.0, NEG).astype(np.float32)
    m1 = np.where(k_idx[:, None] + 128 <= q_idx[None, :], 0.0, NEG).astype(np.float32)
    mask = np.concatenate([m0, m1], axis=1)

    freqs = 1.0 / (THETA ** (np.arange(0, HEAD_DIM, 2, dtype=np.float32) / HEAD_DIM))

    in_maps = []
    for c in range(N_CORES):
        a_sl = slice(CHUNK * c, CHUNK * (c + 1))
        b_sl = slice(CHUNK * (7 - c), CHUNK * (8 - c))
        xT = np.concatenate([x[0, a_sl, :].T, x[1, b_sl, :].T], axis=1)
        pos = np.concatenate([np.arange(CHUNK * c, CHUNK * (c + 1)),
                              np.arange(CHUNK * (7 - c), CHUNK * (8 - c))]).astype(np.float32)
        ph = np.outer(freqs, pos)
        cos64 = np.cos(np.concatenate([ph, ph], axis=0))
        sin64 = np.sin(np.concatenate([ph, ph], axis=0))
        sgn = np.where(np.arange(64) < 32, -1.0, 1.0)[:, None].astype(np.float32)
        cosT = np.concatenate([cos64, cos64], axis=0).astype(np.float32)
        sinT = np.concatenate([sin64 * sgn, sin64 * sgn], axis=0).astype(np.float32)
        in_maps.append({
            "xT": np.ascontiguousarray(xT).astype(ml_dtypes.bfloat16),
            "wqT": wqT, "wkT": wkT, "wvT": wvT, "woT": woT,
            "cosT": cosT.astype(ml_dtypes.bfloat16),
            "sinT": sinT.astype(ml_dtypes.bfloat16), "maskT": mask,
        })
    return in_maps


def kernel(x, wq, wk, wv, wo):
    from concourse.bass_utils import run_bass_kernel_spmd

    if "nc" not in _BUILT:
        _BUILT["nc"] = _build_nc()
    nc = _BUILT["nc"]

    in_maps = _host_inputs(np.asarray(x), np.asarray(wq), np.asarray(wk),
                           np.asarray(wv), np.asarray(wo))
    trace = bool(os.environ.get("BASS_KERNEL_TRACE"))
    res = run_bass_kernel_spmd(nc, in_maps, core_ids=list(range(N_CORES)),
                               trace=trace)
    kernel.last_exec_time_ns = res.exec_time_ns
    kernel.last_results = res

    y = np.empty((B, L, D), dtype=np.float32)
    for c in range(N_CORES):
        yT = res.results[c]["yT"]
        y[0, CHUNK * c: CHUNK * (c + 1), :] = yT[:, 0:CHUNK].T
        y[1, CHUNK * (7 - c): CHUNK * (8 - c), :] = yT[:, CHUNK:].T
    return y
